# revision 1
# baseline (speedup 1.0000x reference)
"""Trainium2 Bass kernel for nn_ContinuousCoprimality.

Per batch row r of two [4096, 16384] fp32 tensors computes
    c_i  = #{x_i[r, :] > 0}
    c_j  = #{x_j[r, :] > 0}
    c_ij = #{(x_i + x_j)[r, :] > 0}
on 8 NeuronCores (rows sharded 512/core); the tiny binary-entropy / E /
threshold tail runs on host in float32, mirroring the reference jnp
arithmetic exactly.

Device-side layout per core: the [512, 16384] shard is a flat buffer viewed
as 16 "megas" of [128 partitions x 4096 fp32] (each partition = one quarter
row, so a mega holds 32 whole rows; DMA is perfectly contiguous).
Per mega:
  DVE:  scr_s = x_i + x_j (fp32, exact);  q_i = (x_i > 0), q_s = (scr_s > 0)
        as bf16 0/1
  ACT:  sg = Sign(x_j); sg = Relu(sg)  -> strict (x_j > 0) indicator, bf16
  PE :  24 matmuls vs a constant block-ones lhsT [128, 32] reduce the
        partition dim (4 quarter-rows -> row) into PSUM [32, 512],
        accumulating the 8 free-dim slices of each quantized tile
  DVE:  tensor_reduce over PSUM free dim -> counts[32, col]
Counts [32, 48] are DMA'd out once; host combines and finishes.

Only production-proven instruction forms are used (plain TT/TS/Activation/
Matmult/TensorReduce) — the fused accum_out variants of TensorScalar /
Activation hit "Too many sync wait commands" in this walrus codegen.
"""

import os as _os

import numpy as np

B, F = 4096, 16384
N_CORES = 8
R = B // N_CORES        # 512 rows per core
P = 128                 # SBUF partitions
W = 4096                # fp32 per partition per mega (quarter row)
QUART = F // W          # 4 partitions per row
ROWS_PER_MEGA = P // QUART  # 32
M = (R * F) // (P * W)  # 16 megas per core
if _os.environ.get("KERNEL_M_OVERRIDE"):
    M = int(_os.environ["KERNEL_M_OVERRIDE"])
LOOPS = int(_os.environ.get("KERNEL_LOOPS", "1"))
NSLICE = W // 512       # matmul free-dim slices per tile

_CACHE = {}
LAST_RESULT = None


def _ones_block_np():
    import ml_dtypes
    w = np.zeros((P, ROWS_PER_MEGA), dtype=np.float32)
    for k in range(P):
        w[k, k // QUART] = 1.0
    return w.astype(ml_dtypes.bfloat16)


def _build_nc():
    import concourse.bass as bass
    import concourse.mybir as mybir
    from concourse.tile import TileContext

    nc = bass.Bass(trn_type="TRN2")
    x_i = nc.dram_tensor("x_i", [R, F], mybir.dt.float32, kind="ExternalInput")
    x_j = nc.dram_tensor("x_j", [R, F], mybir.dt.float32, kind="ExternalInput")
    ones_w = nc.dram_tensor("ones_w", [P, ROWS_PER_MEGA], mybir.dt.bfloat16,
                            kind="ExternalInput")
    cnt_out = nc.dram_tensor("cnt", [ROWS_PER_MEGA, 3 * M], mybir.dt.float32,
                             kind="ExternalOutput")

    xiv = x_i[:, :].flatten().rearrange("(m p f) -> m p f", p=P, f=W)
    xjv = x_j[:, :].flatten().rearrange("(m p f) -> m p f", p=P, f=W)

    gt = mybir.AluOpType.is_gt
    add = mybir.AluOpType.add
    f32 = mybir.dt.float32
    bf16 = mybir.dt.bfloat16

    with TileContext(nc) as tc:
        with tc.tile_pool(name="io", bufs=3) as iop, \
             tc.tile_pool(name="work", bufs=2) as wp, \
             tc.tile_pool(name="small", bufs=1) as sp, \
             tc.tile_pool(name="ps", bufs=2, space="PSUM") as pp:
            ones_t = sp.tile([P, ROWS_PER_MEGA], bf16)
            cnt = sp.tile([ROWS_PER_MEGA, 3 * M], f32)
            nc.sync.dma_start(out=ones_t, in_=ones_w[:, :])
            for m in range(LOOPS * M):
                lp, m = divmod(m, M)
                ti = iop.tile([P, W], f32, tag="ti")
                tj = iop.tile([P, W], f32, tag="tj")
                nc.sync.dma_start(out=ti, in_=xiv[m])
                nc.sync.dma_start(out=tj, in_=xjv[m])

                scr_s = wp.tile([P, W], f32, tag="scrs")
                q_i = wp.tile([P, W], bf16, tag="qi")
                q_s = wp.tile([P, W], bf16, tag="qs")
                sg = wp.tile([P, W], bf16, tag="sg")

                # GPSIMD does the fp32 add (frees the DVE); DVE does the
                # two quantize passes at 2x
                nc.gpsimd.tensor_tensor(scr_s[:, :], ti[:, :], tj[:, :], add)
                nc.vector.tensor_scalar(q_i[:, :], ti[:, :], 0.0, None, gt)
                nc.vector.tensor_scalar(q_s[:, :], scr_s[:, :], 0.0, None, gt)

                # ACT: strict (x_j > 0) as relu(sign(x_j))
                nc.scalar.activation(sg[:, :], tj[:, :],
                                     mybir.ActivationFunctionType.Sign)
                nc.scalar.activation(sg[:, :], sg[:, :],
                                     mybir.ActivationFunctionType.Relu)

                # PE: reduce partitions (4 quarters -> row), accumulate slices
                for t_idx, q in ((0, q_i), (1, sg), (2, q_s)):
                    ps = pp.tile([ROWS_PER_MEGA, 512], f32, tag=f"ps{t_idx}")
                    for s in range(NSLICE):
                        nc.tensor.matmul(
                            ps[:, :],
                            ones_t[:, :],
                            q[:, s * 512:(s + 1) * 512],
                            start=(s == 0),
                            stop=(s == NSLICE - 1),
                        )
                    nc.vector.tensor_reduce(
                        cnt[:, 3 * m + t_idx:3 * m + t_idx + 1],
                        ps[:, :],
                        axis=mybir.AxisListType.X,
                        op=add,
                    )
            nc.sync.dma_start(out=cnt_out[:, :], in_=cnt[:, :])
    return nc


def _split_multi_waits(nc):
    """Walrus in this toolchain encodes exactly one sync-wait per TPB
    instruction (NEURON_ISA_TPB_EVENTS has a single wait slot) and errors
    with "Too many sync wait commands" otherwise.  Tile freely attaches
    several waits to one instruction, so split them: hoist all but the last
    wait onto single-wait Drain carrier instructions inserted just before,
    on the same engine (sequential waits on one engine are equivalent)."""
    import copy as _copy

    import bass_rust
    import concourse.mybir as mb

    nidx = 0
    for f in nc.m.functions:
        new_blocks = []
        for blk in f.blocks:
            new_insts = []
            changed = False
            for ins in blk.instructions:
                si = ins.sync_info
                waits = list(si.on_wait) if si is not None and si.on_wait else []
                upds = list(si.on_update) if si is not None and si.on_update else []
                assert len(upds) <= 1, f"{ins.name}: {len(upds)} sync updates"
                if len(waits) > 1:
                    changed = True
                    for w in waits[:-1]:
                        nidx += 1
                        new_insts.append(mb.InstDrain(
                            name=f"waitsplit-{nidx}",
                            engine=ins.engine,
                            sync_info=bass_rust.SyncInfo(
                                on_wait=[w], on_update=[]),
                        ))
                    ins.sync_info = bass_rust.SyncInfo(
                        on_wait=[waits[-1]], on_update=upds)
                new_insts.append(ins)
            if changed:
                blk.set_instructions_from_list(new_insts) if hasattr(
                    blk, "set_instructions_from_list") else None
                if not hasattr(blk, "set_instructions_from_list"):
                    blk = _copy.replace(blk, instructions=new_insts)
            new_blocks.append(blk)
        if hasattr(f, "set_blocks_from_list"):
            f.set_blocks_from_list(new_blocks)
        else:
            f.blocks = new_blocks
    return nc


def _get_nc():
    if "nc" not in _CACHE:
        _CACHE["nc"] = _split_multi_waits(_build_nc())
    return _CACHE["nc"]


def _counts_from_cnt(cnt):
    """cnt: [32, 3*M] fp32 -> counts [3, R] (exact small ints as f32)."""
    A = cnt.reshape(ROWS_PER_MEGA, M, 3)               # (a, m, t)
    return A.transpose(2, 1, 0).reshape(3, M * ROWS_PER_MEGA)  # row = 32*m + a


def kernel(residue_i, residue_j):
    global LAST_RESULT
    from concourse.bass_utils import run_bass_kernel_spmd

    x_i = np.ascontiguousarray(np.asarray(residue_i, dtype=np.float32))
    x_j = np.ascontiguousarray(np.asarray(residue_j, dtype=np.float32))
    assert x_i.shape == (B, F) and x_j.shape == (B, F)

    nc = _get_nc()
    ones_np = _ones_block_np()
    in_maps = [
        {"x_i": x_i[c * R:(c + 1) * R], "x_j": x_j[c * R:(c + 1) * R],
         "ones_w": ones_np}
        for c in range(N_CORES)
    ]
    res = run_bass_kernel_spmd(nc, in_maps, core_ids=list(range(N_CORES)))
    LAST_RESULT = res

    counts = np.empty((3, B), dtype=np.float32)
    for c in range(N_CORES):
        counts[:, c * R:(c + 1) * R] = _counts_from_cnt(res.results[c]["cnt"])

    # --- entropy on host, float32 to mirror jnp ---
    n = np.float32(F)
    denom = n + np.float32(1e-8)
    c1 = counts.astype(np.float32)            # [3, B]: i, j, ij
    c0 = n - c1
    p0 = c0 / denom
    p1 = c1 / denom

    def term(p):
        return np.where(p > 0, p * np.log2(p + np.float32(1e-10)), np.float32(0.0))

    H = -(term(p0) + term(p1))                # [3, B]: H_i, H_j, H_ij
    E = (H[2] - H[0] - H[1]).astype(np.float32)
    is_co_prime = E >= np.float32(0.0)
    return (is_co_prime, E)



# revision 2
# speedup vs baseline: 1.0479x; 1.0479x over previous
"""Trainium2 Bass kernel for nn_ContinuousCoprimality.

Per batch row r of two [4096, 16384] fp32 tensors computes
    c_i  = #{x_i[r, :] > 0}
    c_j  = #{x_j[r, :] > 0}
    c_ij = #{(x_i + x_j)[r, :] > 0}
on 8 NeuronCores (rows sharded 512/core); the tiny binary-entropy / E /
threshold tail runs on host in float32, mirroring the reference jnp
arithmetic exactly.

Device-side layout per core: each [512, 16384] shard is a flat buffer read
as chunks of [128 partitions x Wc fp32] (each partition = one aligned
Wc-slice of a row; DMA fully contiguous). 15 chunks of Wc=4096 plus 4 tail
chunks of Wc=1024 (smaller tail => shorter post-DMA critical path).

All counting happens on the DVE via fused reduce forms (one pass each,
per-partition counts come out of accum_out for free):
  ts :  o = (x_i > 0),        accum = sum  -> c_i   (op1=add is the reduce)
  ts :  o = (x_j > 0),        accum = sum  -> c_j
  stt:  o = (-x_j < x_i),     accum = sum  -> c_ij  [(x_i+x_j)>0 exactly]
DVE busy ~9.0us per 4096-chunk vs 11.65us of DMA per chunk, so the kernel
runs at the cost-model DMA roofline (64 MiB/core at 360 B/ns = 186.4us)
plus small head/tail. PE/ACT/GPSIMD are unused.

Counts are DMA'd out in two pieces (main chunks early, last chunk tiny)
from the otherwise-idle ACT queue; host combines partition counts into row
counts and finishes the entropy arithmetic.
"""

import numpy as np

B, F = 4096, 16384
N_CORES = 8
R = B // N_CORES        # 512 rows per core
P = 128                 # SBUF partitions
WBIG = 4096             # fp32 per partition per big chunk
NBIG = 15               # big chunks per core
WSM = 1024              # tail chunk width
NSM = 4                 # tail chunks (cover the 16th mega)
NCH = NBIG + NSM        # 19 chunks
NCOL = 3 * NCH          # 57 count columns
NA = 3 * (NCH - 1)      # columns in the early counts DMA

_CACHE = {}
LAST_RESULT = None


def _build_nc():
    import concourse.bass as bass
    import concourse.mybir as mybir
    from concourse.tile import TileContext

    nc = bass.Bass(trn_type="TRN2")
    x_i = nc.dram_tensor("x_i", [R, F], mybir.dt.float32, kind="ExternalInput")
    x_j = nc.dram_tensor("x_j", [R, F], mybir.dt.float32, kind="ExternalInput")
    cnt_out = nc.dram_tensor("cnt", [P, NCOL], mybir.dt.float32,
                             kind="ExternalOutput")

    # flat views: big-chunk grid and small-chunk grid over the same buffer
    xiv_b = x_i[:, :].flatten().rearrange("(m p f) -> m p f", p=P, f=WBIG)
    xjv_b = x_j[:, :].flatten().rearrange("(m p f) -> m p f", p=P, f=WBIG)
    xiv_s = x_i[:, :].flatten().rearrange("(m p f) -> m p f", p=P, f=WSM)
    xjv_s = x_j[:, :].flatten().rearrange("(m p f) -> m p f", p=P, f=WSM)
    SM0 = NBIG * (WBIG // WSM)  # first small-chunk index in the small grid

    add = mybir.AluOpType.add
    gt = mybir.AluOpType.is_gt
    lt = mybir.AluOpType.is_lt
    mult = mybir.AluOpType.mult
    f32 = mybir.dt.float32
    bf16 = mybir.dt.bfloat16

    with TileContext(nc) as tc:
        with tc.tile_pool(name="io", bufs=3) as iop, \
             tc.tile_pool(name="small", bufs=1) as sp:
            o = sp.tile([P, WBIG], bf16)
            cntA = sp.tile([P, NA], f32)
            cntB = sp.tile([P, 3], f32)
            for c in range(NCH):
                if c < NBIG:
                    w, src_i, src_j = WBIG, xiv_b[c], xjv_b[c]
                else:
                    s = SM0 + (c - NBIG)
                    w, src_i, src_j = WSM, xiv_s[s], xjv_s[s]
                cnt = cntA if c < NCH - 1 else cntB
                col = 3 * c if c < NCH - 1 else 0
                ti = iop.tile([P, w], f32, tag=f"ti{w}")
                tj = iop.tile([P, w], f32, tag=f"tj{w}")
                nc.sync.dma_start(out=tj, in_=src_j)
                nc.sync.dma_start(out=ti, in_=src_i)
                nc.vector.tensor_scalar(
                    o[:, :w], tj[:, :], 0.0, None, gt, op1=add,
                    accum_out=cnt[:, col + 1:col + 2])
                nc.vector.tensor_scalar(
                    o[:, :w], ti[:, :], 0.0, None, gt, op1=add,
                    accum_out=cnt[:, col:col + 1])
                nc.vector.scalar_tensor_tensor(
                    o[:, :w], tj[:, :], -1.0, ti[:, :], mult, lt,
                    accum_out=cnt[:, col + 2:col + 3])
            nc.scalar.dma_start(out=cnt_out[:, :NA], in_=cntA[:, :])
            nc.scalar.dma_start(out=cnt_out[:, NA:], in_=cntB[:, :])
    return nc


def _split_multi_waits(nc):
    """Walrus in this toolchain encodes exactly one sync-wait per TPB
    instruction (NEURON_ISA_TPB_EVENTS has a single wait slot) and errors
    with "Too many sync wait commands" otherwise.  Tile freely attaches
    several waits to one instruction, so split them: hoist all but the last
    wait onto single-wait Drain carrier instructions inserted just before,
    on the same engine (sequential waits on one engine are equivalent)."""
    import copy as _copy

    import bass_rust
    import concourse.mybir as mb

    nidx = 0
    for f in nc.m.functions:
        new_blocks = []
        for blk in f.blocks:
            new_insts = []
            changed = False
            for ins in blk.instructions:
                si = ins.sync_info
                waits = list(si.on_wait) if si is not None and si.on_wait else []
                upds = list(si.on_update) if si is not None and si.on_update else []
                assert len(upds) <= 1, f"{ins.name}: {len(upds)} sync updates"
                if len(waits) > 1:
                    changed = True
                    for w in waits[:-1]:
                        nidx += 1
                        new_insts.append(mb.InstDrain(
                            name=f"waitsplit-{nidx}",
                            engine=ins.engine,
                            sync_info=bass_rust.SyncInfo(
                                on_wait=[w], on_update=[]),
                        ))
                    ins.sync_info = bass_rust.SyncInfo(
                        on_wait=[waits[-1]], on_update=upds)
                new_insts.append(ins)
            if changed:
                blk.set_instructions_from_list(new_insts) if hasattr(
                    blk, "set_instructions_from_list") else None
                if not hasattr(blk, "set_instructions_from_list"):
                    blk = _copy.replace(blk, instructions=new_insts)
            new_blocks.append(blk)
        if hasattr(f, "set_blocks_from_list"):
            f.set_blocks_from_list(new_blocks)
        else:
            f.blocks = new_blocks
    return nc


def _get_nc():
    if "nc" not in _CACHE:
        _CACHE["nc"] = _split_multi_waits(_build_nc())
    return _CACHE["nc"]


def _counts_from_cnt(cnt):
    """cnt: [128, 57] fp32 -> counts [3, R] per row (exact ints as f32).

    Big chunk c (Wc=4096): partition p holds quarter q=p%4 of row 32c+p//4.
    Small chunk s (Wc=1024): partition p holds a 1/16 slice of row
    480 + 8s + p//16.
    """
    big = cnt[:, :3 * NBIG].reshape(P, NBIG, 3)
    big = big.reshape(32, 4, NBIG, 3).sum(axis=1)         # [a, c, t]
    out = np.empty((3, R), dtype=np.float32)
    for t in range(3):
        out[t, :32 * NBIG] = big[:, :, t].T.reshape(-1)   # row = 32c + a
    sm = cnt[:, 3 * NBIG:].reshape(P, NSM, 3)
    sm = sm.reshape(8, 16, NSM, 3).sum(axis=1)            # [b, s, t]
    for t in range(3):
        out[t, 32 * NBIG:] = sm[:, :, t].T.reshape(-1)    # row = 480+8s+b
    return out


def kernel(residue_i, residue_j):
    global LAST_RESULT
    from concourse.bass_utils import run_bass_kernel_spmd

    x_i = np.ascontiguousarray(np.asarray(residue_i, dtype=np.float32))
    x_j = np.ascontiguousarray(np.asarray(residue_j, dtype=np.float32))
    assert x_i.shape == (B, F) and x_j.shape == (B, F)

    nc = _get_nc()
    in_maps = [
        {"x_i": x_i[c * R:(c + 1) * R], "x_j": x_j[c * R:(c + 1) * R]}
        for c in range(N_CORES)
    ]
    res = run_bass_kernel_spmd(nc, in_maps, core_ids=list(range(N_CORES)))
    LAST_RESULT = res

    counts = np.empty((3, B), dtype=np.float32)
    for c in range(N_CORES):
        counts[:, c * R:(c + 1) * R] = _counts_from_cnt(res.results[c]["cnt"])

    # --- entropy on host, float32 to mirror jnp ---
    n = np.float32(F)
    denom = n + np.float32(1e-8)
    c1 = counts.astype(np.float32)            # [3, B]: i, j, ij
    c0 = n - c1
    p0 = c0 / denom
    p1 = c1 / denom

    def term(p):
        return np.where(p > 0, p * np.log2(p + np.float32(1e-10)), np.float32(0.0))

    H = -(term(p0) + term(p1))                # [3, B]: H_i, H_j, H_ij
    E = (H[2] - H[0] - H[1]).astype(np.float32)
    is_co_prime = E >= np.float32(0.0)
    return (is_co_prime, E)


# revision 5
# speedup vs baseline: 1.0757x; 1.0265x over previous
"""Trainium2 Bass kernel for nn_ContinuousCoprimality.

Per batch row r of two [4096, 16384] fp32 tensors computes
    c_i  = #{x_i[r, :] > 0}
    c_j  = #{x_j[r, :] > 0}
    c_ij = #{(x_i + x_j)[r, :] > 0}
on 8 NeuronCores (rows sharded 512/core); the tiny binary-entropy / E /
threshold tail runs on host in float32, mirroring the reference jnp
arithmetic exactly.

Device-side layout per core: each [512, 16384] shard is a flat buffer read
as chunks of [128 partitions x Wc fp32] (each partition an aligned Wc-slice
of a row; every DMA fully contiguous). 15 chunks of Wc=4096 then a tapered
tail (3x1024, 512, 256, 256) so the post-DMA critical path is short.

Counting runs on two engines, one pass over each tile (accum_out gives the
per-partition reduction for free):
  DVE ts : o = (x_i > 0)          accum=add -> c_i
  ACT    : oa = Sign(x_j)         accum     -> S_j   (host: c_j=(F+S-z)/2)
  DVE stt: o = ((-x_j) < x_i)     accum=add -> c_ij  [== (x_i+x_j)>0]
DVE ~6.7us + ACT ~4.1us per 4096-chunk vs 11.65us DMA per chunk, so the
kernel runs at the cost-model DMA roofline (64 MiB/core at 360 B/ns =
186.4us) plus ~2.3us head and ~4.4us tail. PE/GPSIMD are unused; oa lives
in PSUM to keep SBUF for deep input double-buffering.

Counts leave in two DMAs (main chunks early from SP, last chunk from ACT's
queue); host folds partition counts into row counts, converts sign-sums
(compensating exact zeros in x_j), and finishes the entropy arithmetic.
"""

import numpy as np

B, F = 4096, 16384
N_CORES = 8
R = B // N_CORES        # 512 rows per core
P = 128                 # SBUF partitions
WMAX = 4096
CHUNKS = [4096] * 15 + [1024] * 3 + [512, 256, 256]
NCH = len(CHUNKS)
NA = 3 * (NCH - 1)      # columns in the early counts DMA
NCOL = 3 * NCH

_CACHE = {}
LAST_RESULT = None


def _build_nc():
    import concourse.bass as bass
    import concourse.mybir as mybir
    from concourse.tile import TileContext

    f32 = mybir.dt.float32
    bf16 = mybir.dt.bfloat16
    nc = bass.Bass(trn_type="TRN2")
    x_i = nc.dram_tensor("x_i", [R, F], f32, kind="ExternalInput")
    x_j = nc.dram_tensor("x_j", [R, F], f32, kind="ExternalInput")
    cnt_out = nc.dram_tensor("cnt", [P, NCOL], f32, kind="ExternalOutput")
    xif = x_i[:, :].flatten()
    xjf = x_j[:, :].flatten()

    add = mybir.AluOpType.add
    gt = mybir.AluOpType.is_gt
    lt = mybir.AluOpType.is_lt
    mult = mybir.AluOpType.mult
    Sign = mybir.ActivationFunctionType.Sign

    with TileContext(nc) as tc:
        with tc.tile_pool(name="io", bufs=4) as iop, \
             tc.tile_pool(name="ios", bufs=4) as iosp, \
             tc.tile_pool(name="small", bufs=1) as sp, \
             tc.tile_pool(name="op", bufs=2) as op_:
            cntA = sp.tile([P, NA], f32)
            cntB = sp.tile([P, 3], f32)
            off = 0
            for c, w in enumerate(CHUNKS):
                src_i = xif[off:off + P * w].rearrange("(p f) -> p f", f=w)
                src_j = xjf[off:off + P * w].rearrange("(p f) -> p f", f=w)
                off += P * w
                cnt = cntA if c < NCH - 1 else cntB
                col = 3 * c if c < NCH - 1 else 0
                pool = iop if w == WMAX else iosp
                sfx = "" if w == WMAX else "s"
                ti = pool.tile([P, w], f32, tag=f"ti{sfx}")
                tj = pool.tile([P, w], f32, tag=f"tj{sfx}")
                nc.sync.dma_start(out=ti[:, :], in_=src_i)
                nc.sync.dma_start(out=tj[:, :], in_=src_j)
                o = op_.tile([P, WMAX], bf16, tag="o")
                oa = op_.tile([P, WMAX], bf16, tag="oa")
                # DVE: c_i (runs while tj is still in flight)
                nc.vector.tensor_scalar(
                    o[:, :w], ti[:, :], 0.0, None, gt, op1=add,
                    accum_out=cnt[:, col:col + 1])
                # ACT: sign-sum of x_j
                nc.scalar.activation(
                    oa[:, :w], tj[:, :], Sign,
                    accum_out=cnt[:, col + 1:col + 2])
                # DVE: c_ij via (-x_j) < x_i
                nc.vector.scalar_tensor_tensor(
                    o[:, :w], tj[:, :], -1.0, ti[:, :], mult, lt,
                    accum_out=cnt[:, col + 2:col + 3])
            nc.sync.dma_start(out=cnt_out[:, :NA], in_=cntA[:, :])
            nc.scalar.dma_start(out=cnt_out[:, NA:], in_=cntB[:, :])
    return nc


def _split_multi_waits(nc):
    """Walrus in this toolchain encodes exactly one sync-wait per TPB
    instruction (NEURON_ISA_TPB_EVENTS has a single wait slot) and errors
    with "Too many sync wait commands" otherwise.  Tile freely attaches
    several waits to one instruction, so split them: hoist all but the last
    wait onto single-wait Drain carrier instructions inserted just before,
    on the same engine (sequential waits on one engine are equivalent)."""
    import copy as _copy

    import bass_rust
    import concourse.mybir as mb

    nidx = 0
    for f in nc.m.functions:
        new_blocks = []
        for blk in f.blocks:
            new_insts = []
            changed = False
            for ins in blk.instructions:
                si = ins.sync_info
                waits = list(si.on_wait) if si is not None and si.on_wait else []
                upds = list(si.on_update) if si is not None and si.on_update else []
                assert len(upds) <= 1, f"{ins.name}: {len(upds)} sync updates"
                if len(waits) > 1:
                    changed = True
                    for w in waits[:-1]:
                        nidx += 1
                        new_insts.append(mb.InstDrain(
                            name=f"waitsplit-{nidx}",
                            engine=ins.engine,
                            sync_info=bass_rust.SyncInfo(
                                on_wait=[w], on_update=[]),
                        ))
                    ins.sync_info = bass_rust.SyncInfo(
                        on_wait=[waits[-1]], on_update=upds)
                new_insts.append(ins)
            if changed:
                blk.set_instructions_from_list(new_insts) if hasattr(
                    blk, "set_instructions_from_list") else None
                if not hasattr(blk, "set_instructions_from_list"):
                    blk = _copy.replace(blk, instructions=new_insts)
            new_blocks.append(blk)
        if hasattr(f, "set_blocks_from_list"):
            f.set_blocks_from_list(new_blocks)
        else:
            f.blocks = new_blocks
    return nc


def _get_nc():
    if "nc" not in _CACHE:
        _CACHE["nc"] = _split_multi_waits(_build_nc())
    return _CACHE["nc"]


def _counts_from_cnt(cnt):
    """cnt: [128, NCOL] fp32 -> (c_i, S_j, c_ij) summed per row, [3, R].

    Chunk c (width w) starts at a row boundary (128*w is a multiple of
    F for every width used) and covers 128*w/F rows with F/w partitions
    per row.
    """
    out = np.zeros((3, R), dtype=np.float64)
    off = 0
    for c, w in enumerate(CHUNKS):
        col = 3 * c if c < NCH - 1 else NA
        row0 = off // F
        nrows = (P * w) // F
        ppr = F // w
        for t in range(3):
            out[t, row0:row0 + nrows] += (
                cnt[:, col + t].reshape(nrows, ppr).sum(axis=1))
        off += P * w
    return out


def kernel(residue_i, residue_j):
    global LAST_RESULT
    from concourse.bass_utils import run_bass_kernel_spmd

    x_i = np.ascontiguousarray(np.asarray(residue_i, dtype=np.float32))
    x_j = np.ascontiguousarray(np.asarray(residue_j, dtype=np.float32))
    assert x_i.shape == (B, F) and x_j.shape == (B, F)

    nc = _get_nc()
    in_maps = [
        {"x_i": x_i[c * R:(c + 1) * R], "x_j": x_j[c * R:(c + 1) * R]}
        for c in range(N_CORES)
    ]
    res = run_bass_kernel_spmd(nc, in_maps, core_ids=list(range(N_CORES)))
    LAST_RESULT = res

    raw = np.empty((3, B), dtype=np.float64)
    for c in range(N_CORES):
        raw[:, c * R:(c + 1) * R] = _counts_from_cnt(res.results[c]["cnt"])

    # c_j from sign-sums: c1 = (F + S - z)/2, z = exact zeros in the row
    z = (x_j == 0).sum(axis=1)
    counts = np.empty((3, B), dtype=np.float32)
    counts[0] = raw[0]
    counts[1] = (np.float64(F) + raw[1] - z) / 2.0
    counts[2] = raw[2]

    # --- entropy on host, float32 to mirror jnp ---
    n = np.float32(F)
    denom = n + np.float32(1e-8)
    c1 = counts.astype(np.float32)            # [3, B]: i, j, ij
    c0 = n - c1
    p0 = c0 / denom
    p1 = c1 / denom

    def term(p):
        return np.where(p > 0, p * np.log2(p + np.float32(1e-10)), np.float32(0.0))

    H = -(term(p0) + term(p1))                # [3, B]: H_i, H_j, H_ij
    E = (H[2] - H[0] - H[1]).astype(np.float32)
    is_co_prime = E >= np.float32(0.0)
    return (is_co_prime, E)
